# revision 22
# baseline (speedup 1.0000x reference)
"""Trainium2 Bass kernel for nn_GAT_Decoder (one decode step of a GAT decoder).

Strategy (per spec sharding hint): pure data parallel, batch sharded 8 ways
(32 batch elements per core), weights replicated.

v2 design notes vs the earlier kernel:
- E streams in bf16, and E^T is transposed on the HOST and DMA'd (bf16), so
  the PE does no [128,128] E^T transposes at all.  Total E DMA (E + E^T in
  bf16) equals the old f32 E alone.
- Masked rows are compacted out host-side; padding rows of E / padding cols
  of E^T are ZERO, so padded compat entries are exactly 0 and contribute
  exp(0)=1 to softmax sums.  The host passes npad = n_c - count per batch and
  the kernel subtracts it from each softmax denominator.  No mask tensors on
  device at all.
- Batches are processed in quads (4).  All small-M matmuls (compat M=8,
  ctx M=8, compat2 M=1) are col-tiled via tile_position=(0,32j) so the four
  batches' matmuls run concurrently on disjoint PE column groups.
- o = ctx @ Wv (per-head diag blocks) and phat = G^T @ oT contract against
  *fixed* weights, so they run once per quad with the weight as stationary
  and the quad's 4 batches stacked on the moving free dim.
- The pointer softmax (tanh/exp/normalize) is computed once per core on a
  packed [32, n_c] tile instead of per-batch rows.
"""

import numpy as np

B, N, D, H = 256, 1000, 512, 8
HD = D // H
NCORES = 8
BPC = B // NCORES          # batches per core
QUAD = 4                   # batches per quad (PE column-group packing)
ND = D // 128              # 4 contraction chunks

NORM_MHA = float(1.0 / np.sqrt(HD))
NORM_PTR = float(1.0 / np.sqrt(D))
DEBUG = False              # adds intermediate DRAM dumps (quad 0)


def _build(n_c, bpc):
    from concourse import bacc
    import concourse.mybir as mybir
    import concourse.tile as tile
    from concourse.masks import make_identity

    dt = mybir.dt
    AF = mybir.ActivationFunctionType
    ALU = mybir.AluOpType
    f32 = dt.float32
    bf16 = dt.bfloat16
    pdt = dt.float32r          # phase-0 dtype

    nt = n_c // 128
    nh = n_c // 2
    nquad = bpc // QUAD

    nc = bacc.Bacc("TRN2", target_bir_lowering=False, debug=False)

    # ---- DRAM I/O ----
    Ec_d = nc.dram_tensor("Ec", [bpc, n_c, D], bf16, kind="ExternalInput")
    EcT_d = nc.dram_tensor("EcT", [bpc, D, n_c], bf16, kind="ExternalInput")
    wfc_d = nc.dram_tensor("W_fc", [D + 1, D], pdt, kind="ExternalInput")
    wfc1_d = nc.dram_tensor("W_fc1", [D, D], pdt, kind="ExternalInput")
    wq_d = nc.dram_tensor("Wq", [D, D], pdt, kind="ExternalInput")
    wkT_d = nc.dram_tensor("WkT", [D, D], pdt, kind="ExternalInput")
    wv_d = nc.dram_tensor("Wv", [D, D], bf16, kind="ExternalInput")
    woT_d = nc.dram_tensor("WoT", [D, D], pdt, kind="ExternalInput")
    wpT_d = nc.dram_tensor("WpT", [D, D], pdt, kind="ExternalInput")
    wlastT_d = nc.dram_tensor("wlastT", [128, ND], f32, kind="ExternalInput")
    e0T_d = nc.dram_tensor("E0T", [D, bpc], pdt, kind="ExternalInput")
    poolT_d = nc.dram_tensor("poolT", [D, bpc], pdt, kind="ExternalInput")
    dcrep_d = nc.dram_tensor("dcrep", [128, bpc], f32, kind="ExternalInput")
    npadq_d = nc.dram_tensor("npadq", [nquad, 128, 1], f32, kind="ExternalInput")
    npad32_d = nc.dram_tensor("npad32", [bpc, 1], f32, kind="ExternalInput")
    out_d = nc.dram_tensor("scores", [bpc, n_c], f32, kind="ExternalOutput")
    if DEBUG:
        nquad_ = bpc // QUAD
        dbg_ex_d = nc.dram_tensor("dbg_ex", [nquad_, 128, n_c], f32, kind="ExternalOutput")
        dbg_eT_d = nc.dram_tensor("dbg_eT", [nquad_, 128, nt, 128], f32, kind="ExternalOutput")
        dbg_ctx_d = nc.dram_tensor("dbg_ctx", [nquad_, 128, D], f32, kind="ExternalOutput")
        dbg_oT_d = nc.dram_tensor("dbg_oT", [128, ND, nquad_, QUAD], f32, kind="ExternalOutput")
        dbg_ph_d = nc.dram_tensor("dbg_ph", [nquad_, 128, ND, QUAD], f32, kind="ExternalOutput")
        dbg_x_d = nc.dram_tensor("dbg_x", [bpc, n_c], f32, kind="ExternalOutput")
        dbg_s_d = nc.dram_tensor("dbg_s", [128, 2], f32, kind="ExternalOutput")

    def w_ap(d):  # [512,512] dram -> [128, 4, 512]
        return d[0:D, :].rearrange("(c p) d -> p c d", p=128)

    with tile.TileContext(nc) as tc:
        with tc.tile_pool(name="const", bufs=1) as constp, \
             tc.tile_pool(name="wmain", bufs=1) as wmain:
            ident = constp.tile([128, 128], f32, tag="ident")
            make_identity(nc, ident[:])
            identb = constp.tile([128, 128], bf16, tag="identb")
            nc.vector.tensor_copy(identb[:], ident[:])

            # persistent
            wv_t = wmain.tile([128, ND, D], bf16, tag="wv")
            gt_t = wmain.tile([128, ND, D], bf16, tag="gt")
            qhatT = wmain.tile([128, ND, bpc, H], bf16, tag="qhatT")
            ctxT_g = wmain.tile([128, ND, nquad, 32], bf16, tag="ctxTg")
            oT_g = wmain.tile([128, ND, nquad, QUAD], bf16, tag="oTg")
            x_all = wmain.tile([bpc, n_c], f32, tag="xall")
            npad32_t = wmain.tile([bpc, 1], f32, tag="npad32")
            nc.gpsimd.dma_start(wv_t[:], w_ap(wv_d))
            nc.gpsimd.dma_start(npad32_t[:], npad32_d[:])

            # ---------- phase 0 (f32r): qhatT, gt, state-derived queries ----
            with tc.tile_pool(name="w0", bufs=1) as w0, \
                 tc.tile_pool(name="ps0", bufs=2, space="PSUM") as ps0:
                wfc_t = w0.tile([128, ND, D], pdt, tag="wfc")
                wfc1_t = w0.tile([128, ND, D], pdt, tag="wfc1")
                wq_t = w0.tile([128, ND, D], pdt, tag="wq")
                wkT_t = w0.tile([128, ND, D], pdt, tag="wkT")
                woT_t = w0.tile([128, ND, D], pdt, tag="woT")
                wpT_t = w0.tile([128, ND, D], pdt, tag="wpT")
                wlast_t = w0.tile([128, ND], f32, tag="wlast")
                e0T_t = w0.tile([128, ND, bpc], pdt, tag="e0T")
                poolT_t = w0.tile([128, ND, bpc], pdt, tag="poolT")
                dcrep_t = w0.tile([128, bpc], f32, tag="dcrep")
                stateT = w0.tile([128, ND, bpc], pdt, tag="stateT")
                qT_t = w0.tile([128, ND, bpc], pdt, tag="qT")
                nc.gpsimd.dma_start(wfc_t[:], w_ap(wfc_d))
                nc.gpsimd.dma_start(wfc1_t[:], w_ap(wfc1_d))
                nc.gpsimd.dma_start(wq_t[:], w_ap(wq_d))
                nc.gpsimd.dma_start(wkT_t[:], w_ap(wkT_d))
                nc.gpsimd.dma_start(woT_t[:], w_ap(woT_d))
                nc.gpsimd.dma_start(wpT_t[:], w_ap(wpT_d))
                nc.gpsimd.dma_start(wlast_t[:], wlastT_d[:])
                nc.gpsimd.dma_start(e0T_t[:], e0T_d[:].rearrange("(c p) b -> p c b", p=128))
                nc.gpsimd.dma_start(poolT_t[:], poolT_d[:].rearrange("(c p) b -> p c b", p=128))
                nc.gpsimd.dma_start(dcrep_t[:], dcrep_d[:])

                # G^T = WoT.T @ WpT, scaled by norm_ptr
                for c in range(ND):
                    g_ps = ps0.tile([128, D], f32, tag="g_ps")
                    for kc in range(ND):
                        nc.tensor.matmul(g_ps[:], woT_t[:, kc, 128 * c:128 * (c + 1)],
                                         wpT_t[:, kc, :], start=(kc == 0), stop=(kc == ND - 1))
                    nc.scalar.mul(gt_t[:, c, :], g_ps[:], NORM_PTR)

                # stateT
                for c in range(ND):
                    st_ps = ps0.tile([128, bpc], f32, tag="st_ps")
                    for kc in range(ND):
                        nc.tensor.matmul(st_ps[:], wfc_t[:, kc, 128 * c:128 * (c + 1)],
                                         e0T_t[:, kc, :], start=(kc == 0), stop=False)
                    for kc in range(ND):
                        nc.tensor.matmul(st_ps[:], wfc1_t[:, kc, 128 * c:128 * (c + 1)],
                                         poolT_t[:, kc, :], start=False, stop=(kc == ND - 1))
                    nc.vector.scalar_tensor_tensor(
                        stateT[:, c, :], dcrep_t[:], wlast_t[:, c:c + 1], st_ps[:],
                        op0=ALU.mult, op1=ALU.add)

                # QT (scaled by norm_mha)
                for c in range(ND):
                    q_ps = ps0.tile([128, bpc], f32, tag="q_ps")
                    for kc in range(ND):
                        nc.tensor.matmul(q_ps[:], wq_t[:, kc, 128 * c:128 * (c + 1)],
                                         stateT[:, kc, :], start=(kc == 0), stop=(kc == ND - 1))
                    nc.scalar.mul(qT_t[:, c, :], q_ps[:], NORM_MHA)

                # qhatT[d, c, b, h] = (Wk_h^T q_b)_d  (64-row head blocks packed)
                for h in range(H):
                    pb = 64 * (h % 2)
                    for c in range(ND):
                        qq = ps0.tile([128, bpc], f32, tag="qq")
                        nc.tensor.matmul(
                            qq[:], wkT_t[pb:pb + 64, h // 2, 128 * c:128 * (c + 1)],
                            qT_t[pb:pb + 64, h // 2, :], start=True, stop=True)
                        nc.vector.tensor_copy(qhatT[:, c, :, h], qq[:])

            # ---------- main loop: quads of 4 batches ----------
            with tc.tile_pool(name="epool", bufs=3) as epool, \
                 tc.tile_pool(name="etpool", bufs=5) as etpool, \
                 tc.tile_pool(name="expool", bufs=3) as expool, \
                 tc.tile_pool(name="smpool", bufs=2) as smpool, \
                 tc.tile_pool(name="small", bufs=4) as smallp, \
                 tc.tile_pool(name="xsb", bufs=2) as xsbp, \
                 tc.tile_pool(name="cps", bufs=2, space="PSUM") as cps, \
                 tc.tile_pool(name="ctxps", bufs=1, space="PSUM") as ctxps, \
                 tc.tile_pool(name="tpsA", bufs=1, space="PSUM") as tpsA, \
                 tc.tile_pool(name="tpsB", bufs=1, space="PSUM") as tpsB, \
                 tc.tile_pool(name="otps", bufs=1, space="PSUM") as otps:
                st = {}

                def emit_dma(q):
                    et4 = etpool.tile([128, QUAD, ND, n_c], bf16, tag="ET")
                    nc.sync.dma_start(
                        et4[:], EcT_d[QUAD * q:QUAD * (q + 1)].rearrange(
                            "b (c p) n -> p b c n", p=128))
                    e4 = epool.tile([128, QUAD, nt, D], bf16, tag="E")
                    nc.sync.dma_start(
                        e4[:], Ec_d[QUAD * q:QUAD * (q + 1)].rearrange(
                            "b (t p) d -> p b t d", p=128))
                    npad_t = smallp.tile([128, 1], f32, tag="npad")
                    nc.sync.dma_start(npad_t[:], npadq_d[q])
                    st[q] = dict(et=[et4[:, j] for j in range(QUAD)],
                                 e=[e4[:, j] for j in range(QUAD)],
                                 npad=npad_t)

                def emit_memset(q):
                    ex4b = expool.tile([128, n_c], bf16, tag="ex4b")
                    # garbage rows (32j+8 .. 32j+32) are never written by the
                    # exp stage, so zeroing each ring buffer once suffices
                    if q < 3:
                        nc.gpsimd.memset(ex4b[:], 0.0)
                    st[q]['ex'] = ex4b

                def emit_compat(q):
                    # [128, 2, 512] so each half's accumulation region is
                    # bank-aligned (a matmul output must not cross a PSUM bank)
                    cp = cps.tile([128, 2, 512], f32, tag="cp")
                    ets = st[q]['et']
                    for half in range(2):
                        for j in range(QUAD):
                            for c in range(ND):
                                nc.tensor.matmul(
                                    cp[32 * j:32 * j + 8, half, 0:nh],
                                    qhatT[:, c, QUAD * q + j, :],
                                    ets[j][:, c, half * nh:(half + 1) * nh],
                                    start=(c == 0), stop=(c == ND - 1),
                                    tile_position=(0, 32 * j))
                    st[q]['cp'] = cp

                def emit_exp(q):
                    cp, ex4b = st[q]['cp'], st[q]['ex']
                    s_t = smallp.tile([128, 1], f32, tag="s")
                    for j in range(QUAD):
                        nc.scalar.activation(
                            ex4b[32 * j:32 * j + 8, :], cp[32 * j:32 * j + 8, :, 0:nh],
                            AF.Exp, bias=0.0, scale=1.0,
                            accum_out=s_t[32 * j:32 * j + 8, :])
                    st[q]['s'] = s_t

                def emit_expT(q):
                    ex4b, s_t, npad_t = st[q]['ex'], st[q]['s'], st[q]['npad']
                    # softmax denominators: 1 / (sum - npad)
                    r_t = smallp.tile([128, 1], f32, tag="r")
                    nc.vector.tensor_sub(r_t[:], s_t[:], npad_t[:])
                    nc.vector.reciprocal(r_t[:], r_t[:])
                    st[q]['r'] = r_t
                    tpT = tpsA.tile([128, nt, 128], bf16, tag="tpT")
                    for t in range(nt):
                        nc.tensor.transpose(
                            tpT[:, t, :], ex4b[:, 128 * t:128 * (t + 1)], identb[:])
                    expT4b = smpool.tile([128, nt, 128], bf16, tag="expT")
                    nc.vector.tensor_copy(expT4b[:], tpT[:])
                    st[q]['expT'] = expT4b

                def emit_ctx(q):
                    expT4b, es = st[q]['expT'], st[q]['e']
                    ctxp = ctxps.tile([128, D], f32, tag="ctxp")
                    nc.vector.memset(ctxp[:], 0.0)
                    for j in range(QUAD):
                        for t in range(nt):
                            nc.tensor.matmul(
                                ctxp[32 * j:32 * j + 8, :],
                                expT4b[:, t, 32 * j:32 * j + 8],
                                es[j][:, t, :],
                                start=(t == 0), stop=(t == nt - 1),
                                tile_position=(0, 32 * j))
                    st[q]['ctxp'] = ctxp

                def emit_ctxcopy(q):
                    ctxp, r_t = st[q]['ctxp'], st[q]['r']
                    ctx4b = smpool.tile([128, D], bf16, tag="ctx4b")
                    nc.scalar.activation(ctx4b[:], ctxp[:], AF.Copy,
                                         bias=0.0, scale=r_t[:, 0:1])
                    st[q]['ctx'] = ctx4b

                def emit_ctxT(q):
                    ctx4b = st[q]['ctx']
                    tpC = tpsB.tile([128, ND, 128], bf16, tag="tpC")
                    for c in range(ND):
                        nc.tensor.transpose(
                            tpC[:, c, :], ctx4b[:, 128 * c:128 * (c + 1)], identb[:])
                    # gather valid cols m=32j+h -> ctxT_g[:, c, q, 8j+h]
                    nc.vector.tensor_copy(
                        ctxT_g[:, :, q, :].rearrange("p c (j h) -> p c j h", j=QUAD),
                        tpC[:, :, :].rearrange("p c (j x) -> p c j x", j=QUAD)[:, :, :, 0:8])

                def emit_o(q):
                    # oT[64h+k, j] = sum_d ctx[j,h,d] Wv[d, 64h+k]
                    oTp = otps.tile([128, ND, QUAD], f32, tag="op")
                    rhs = ctxT_g[:, :, q, :].rearrange("p c (j h) -> p c j h", h=8)
                    for cc in range(ND):
                        for h in (2 * cc, 2 * cc + 1):
                            pb = 64 * (h % 2)
                            for c in range(ND):
                                nc.tensor.matmul(
                                    oTp[pb:pb + 64, cc, :],
                                    wv_t[:, c, 64 * h:64 * (h + 1)],
                                    rhs[:, c, :, h],
                                    start=(c == 0), stop=(c == ND - 1),
                                    tile_position=(0, pb))
                    nc.vector.tensor_copy(oT_g[:, :, q, :], oTp[:])

                def emit_phat(q):
                    php = otps.tile([128, ND, QUAD], f32, tag="op")
                    for c2 in range(ND):
                        for c in range(ND):
                            nc.tensor.matmul(
                                php[:, c2, :],
                                gt_t[:, c, 128 * c2:128 * (c2 + 1)],
                                oT_g[:, c, q, :],
                                start=(c == 0), stop=(c == ND - 1))
                    phatT_q = smallp.tile([128, ND, QUAD], bf16, tag="phatT")
                    nc.vector.tensor_copy(phatT_q[:], php[:])
                    st[q]['phat'] = phatT_q

                def emit_c2(q):
                    phatT_q, ets = st[q]['phat'], st[q]['et']
                    cp2 = cps.tile([128, 2, 512], f32, tag="cp")
                    for half in range(2):
                        for j in range(QUAD):
                            for c in range(ND):
                                nc.tensor.matmul(
                                    cp2[32 * j:32 * j + 1, half, 0:nh],
                                    phatT_q[:, c, j:j + 1],
                                    ets[j][:, c, half * nh:(half + 1) * nh],
                                    start=(c == 0), stop=(c == ND - 1),
                                    tile_position=(0, 32 * j))
                    x_sb = xsbp.tile([128, 2, nh], f32, tag="xsb")
                    for j in range(QUAD):
                        eng = nc.scalar.copy if j % 2 == 0 else nc.vector.tensor_copy
                        eng(x_sb[32 * j:32 * j + 1, :, :], cp2[32 * j:32 * j + 1, :, 0:nh])
                    nc.gpsimd.dma_start(
                        x_all[QUAD * q:QUAD * (q + 1), :],
                        x_sb[:, :, :].rearrange("(j r) a b -> j r (a b)", r=32)[:, 0, :])
                    del st[q]

                def emit_debug(q):
                    if not DEBUG:
                        return
                    if q == 0:
                        d5 = wmain.tile([128, 2], f32, tag="d5")
                        nc.vector.tensor_copy(d5[:, 0:1], st[q]['s'][:])
                        nc.vector.tensor_copy(d5[:, 1:2], st[q]['r'][:])
                        nc.sync.dma_start(dbg_s_d[:], d5[:])
                    d1 = wmain.tile([128, n_c], f32, tag="d1")
                    nc.scalar.copy(d1[:], st[q]['ex'][:])
                    nc.sync.dma_start(dbg_ex_d[q], d1[:])
                    d6 = wmain.tile([128, nt, 128], f32, tag="d6")
                    nc.scalar.copy(d6[:], st[q]['expT'][:])
                    nc.sync.dma_start(dbg_eT_d[q], d6[:])
                    d2 = wmain.tile([128, D], f32, tag="d2")
                    nc.scalar.copy(d2[:], st[q]['ctx'][:])
                    nc.sync.dma_start(dbg_ctx_d[q], d2[:])
                    d4 = wmain.tile([128, ND, QUAD], f32, tag="d4")
                    nc.scalar.copy(d4[:], st[q]['phat'][:])
                    nc.sync.dma_start(dbg_ph_d[q], d4[:])

                def emit_ptr():
                    if DEBUG:
                        d3 = wmain.tile([128, ND, nquad, QUAD], f32, tag="d3")
                        nc.vector.tensor_copy(d3[:], oT_g[:])
                        nc.sync.dma_start(dbg_oT_d[:], d3[:])
                        nc.sync.dma_start(dbg_x_d[:], x_all[:])
                    # x_all holds compat2 for all 32 batches; softmax(10*tanh(x))
                    th = wmain.tile([bpc, n_c], f32, tag="th")
                    nc.scalar.activation(th[:], x_all[:], AF.Tanh)
                    e2 = wmain.tile([bpc, n_c], f32, tag="e2")
                    s2 = wmain.tile([bpc, 1], f32, tag="s2")
                    nc.scalar.activation(e2[:], th[:], AF.Exp,
                                         bias=0.0, scale=10.0, accum_out=s2[:])
                    r2 = wmain.tile([bpc, 1], f32, tag="r2")
                    nc.vector.tensor_sub(r2[:], s2[:], npad32_t[:])
                    nc.vector.reciprocal(r2[:], r2[:])
                    sc = wmain.tile([bpc, n_c], f32, tag="sc")
                    nc.vector.tensor_scalar_mul(sc[:], e2[:], r2[:])
                    nc.sync.dma_start(out_d[:], sc[:])

                nquad_r = nquad
                emit_dma(0)
                for r in range(nquad_r + 2):
                    if r + 1 < nquad_r:
                        emit_dma(r + 1)
                    if r < nquad_r:
                        emit_memset(r)
                    if 1 <= r <= nquad_r:
                        emit_expT(r - 1)
                    if r < nquad_r:
                        emit_compat(r)
                        emit_exp(r)
                    if 1 <= r <= nquad_r:
                        emit_ctx(r - 1)
                        emit_ctxcopy(r - 1)
                    if 2 <= r <= nquad_r + 1:
                        emit_ctxT(r - 2)
                        emit_o(r - 2)
                        emit_phat(r - 2)
                        emit_debug(r - 2)
                        emit_c2(r - 2)
                emit_ptr()

    nc.finalize()
    return nc


def _host_prep(inputs, n_c=None):
    E = np.ascontiguousarray(inputs['encoder_inputs'], dtype=np.float32)
    mask = np.asarray(inputs['mask'])
    unm = (mask == 0)
    counts = unm.sum(axis=1).astype(np.int64)
    if n_c is None:
        n_c = max(512, int(np.ceil(counts.max() / 128) * 128))
    idx = np.zeros((B, n_c), dtype=np.int64)
    for b in range(B):
        ii = np.nonzero(unm[b])[0]
        k = min(len(ii), n_c)
        idx[b, :k] = ii[:k]
    Ec = np.take_along_axis(E, idx[:, :, None], axis=1)   # [B, n_c, D]
    # zero the padding rows so padded compat entries are exactly 0
    pad = np.arange(n_c)[None, :] >= counts[:, None]      # [B, n_c]
    Ec[pad] = 0.0
    return Ec, idx, counts, n_c


def _in_maps(inputs, Ec, counts, n_c, bpc=BPC):
    import ml_dtypes
    bf16 = ml_dtypes.bfloat16
    nquad = bpc // QUAD
    W_fc = np.asarray(inputs['W_fc'], dtype=np.float32)
    wlastT = np.ascontiguousarray(W_fc[D].reshape(ND, 128).T)        # [128, 4]
    wkT = np.ascontiguousarray(np.asarray(inputs['Wk_mha']).T)
    woT = np.ascontiguousarray(np.asarray(inputs['Wo']).T)
    wpT = np.ascontiguousarray(np.asarray(inputs['Wk_ptr']).T)
    pool = np.asarray(inputs['pool'], dtype=np.float32)
    dc = np.asarray(inputs['dynamic_capacity'], dtype=np.float32)
    Ecb = Ec.astype(bf16)
    npad = (n_c - counts).astype(np.float32)
    maps = []
    for i in range(NCORES):
        b0 = i * bpc
        npadq = np.repeat(npad[b0:b0 + bpc].reshape(nquad, QUAD), 32, axis=1)
        m = {
            "Ec": np.ascontiguousarray(Ecb[b0:b0 + bpc]),
            "EcT": np.ascontiguousarray(Ecb[b0:b0 + bpc].transpose(0, 2, 1)),
            "W_fc": W_fc,
            "W_fc1": np.asarray(inputs['W_fc1'], dtype=np.float32),
            "Wq": np.asarray(inputs['Wq'], dtype=np.float32),
            "WkT": wkT,
            "Wv": np.asarray(inputs['Wv'], dtype=np.float32).astype(bf16),
            "WoT": woT,
            "WpT": wpT,
            "wlastT": wlastT,
            "E0T": np.ascontiguousarray(Ec[b0:b0 + bpc, 0, :].T),
            "poolT": np.ascontiguousarray(pool[b0:b0 + bpc].T),
            "dcrep": np.ascontiguousarray(np.broadcast_to(dc[b0:b0 + bpc, 0], (128, bpc))),
            "npadq": np.ascontiguousarray(npadq.reshape(nquad, 128, 1)),
            "npad32": np.ascontiguousarray(npad[b0:b0 + bpc].reshape(bpc, 1)),
        }
        maps.append(m)
    return maps


_cache = {}


def _get_nc(n_c, bpc):
    key = (n_c, bpc)
    if key not in _cache:
        _cache[key] = _build(n_c, bpc)
    return _cache[key]


def run(inputs, trace=False, **_ignored):
    from concourse.bass_utils import run_bass_kernel_spmd
    Ec, idx, counts, n_c = _host_prep(inputs)
    nc = _get_nc(n_c, BPC)
    maps = _in_maps(inputs, Ec, counts, n_c, BPC)
    res = run_bass_kernel_spmd(nc, maps, list(range(NCORES)), trace=trace)
    scores = np.zeros((B, N), dtype=np.float32)
    for i in range(NCORES):
        sc = res.results[i]["scores"]
        for j in range(BPC):
            b = i * BPC + j
            c = counts[b]
            scores[b, idx[b, :c]] = sc[j, :c]
    return scores, res


def kernel(**inputs) -> np.ndarray:
    scores, _ = run(inputs, trace=False)
    return scores
